# revision 13
# baseline (speedup 1.0000x reference)
"""Multi-head causal attention (B=1, T=4096, D=768, H=12) on 8 trn2 cores.

Sharding: 16 uniform head-slots (2 per core), 12 real heads + 4 dummy
(zero-weight) slots.  Every core runs the IDENTICAL program (SPMD); cores
differ only in the weight data they receive.  Each core computes, for its
two head-slots, the full causal attention over all 4096 tokens plus that
slot-pair's partial output projection.  The host sums the 8 partial
[768, 4096] bf16 outputs, transposes, and adds the output bias.

v3 layout (per core):
  xT    [768, 4096] bf16   x transposed (host supplies), DMA'd per tok-tile
  QT/KT [128, 4096]  bf16  partitions 0:64 slot A dims, 64:128 slot B
  VT    [128, 4096]  bf16  V pre-transpose, then PE-transposed into
  V2    [128, 32*256] bf16 per key chunk c: [V_A | ones x128 | V_B]
                           -> AV matmul lhsT [V_A|ones64] / [ones64|V_B]
                           gives AV rows plus 64x-replicated denominators
  scores in [128 keys, 256 queries] chunks; 6 chunks per ACT exp group;
  av PSUM [128, 512] = ONE bank (A cols 0:256, B cols 256:512), double
  buffered so query tiles overlap.  Normalize: reciprocal_approx_fast of
  the replicated denom block, DRAM-bounce partition broadcast, division
  fused into the PSUM->SBUF copy, merged (k=128) out-proj, bf16 output.
  The issue order is software-pipelined: scores of group i+1 are issued
  before exp/AV of group i so the PE never sits behind the ACT exp.
"""

import math
import numpy as np
import ml_dtypes
from contextlib import ExitStack

import concourse.bass as bass
import concourse.bacc as bacc
import concourse.mybir as mybir
import concourse.tile as tile
from concourse.bass_utils import run_bass_kernel_spmd

BF16 = mybir.dt.bfloat16
F32 = mybir.dt.float32
AF = mybir.ActivationFunctionType

T = 4096
D_MODEL = 768
HEAD_DIM = 64
N_HEADS = 12
N_CORES = 8
QT = 512                  # query tile width
KC = 128                  # key chunk (psum partition dim)
GRP = 2                   # score chunk-jobs per exp group -> ACT free dim 1024
NPAT = 4                  # straddle mask patterns per query tile (QT//KC)
NQT = T // QT             # 16 query tiles
CCH = D_MODEL // 128      # 6 contraction chunks
TOKT = 512                # token tile for projections
NTOKT = T // TOKT
VST = 256                 # V2 stride per 128-key chunk

_PROGRAM_CACHE = {}


def build_program():
    nc = bacc.Bacc(None)

    xT_d = nc.declare_dram_parameter("xT", [D_MODEL, T], BF16, isOutput=False)
    w_d = nc.declare_dram_parameter("wqkv", [3, D_MODEL, 128], BF16, isOutput=False)
    b_d = nc.declare_dram_parameter("bqkv", [128, 3], F32, isOutput=False)
    wo_d = nc.declare_dram_parameter("wo2", [128, D_MODEL], BF16, isOutput=False)
    mk_d = nc.declare_dram_parameter("masks", [NPAT, 128, QT], BF16, isOutput=False)
    id_d = nc.declare_dram_parameter("ident", [128, 128], BF16, isOutput=False)
    outT_d = nc.declare_dram_parameter("outT", [D_MODEL, T], BF16, isOutput=True)

    with tile.TileContext(nc) as tc, ExitStack() as ctx:
        consts = ctx.enter_context(tc.tile_pool(name="consts", bufs=1))
        big = ctx.enter_context(tc.tile_pool(name="big", bufs=1))
        ptp = ctx.enter_context(tc.tile_pool(name="ptp", bufs=3))
        rp = ctx.enter_context(tc.tile_pool(name="rp", bufs=2))
        hvp = ctx.enter_context(tc.tile_pool(name="hvp", bufs=2))
        osb = ctx.enter_context(tc.tile_pool(name="osb", bufs=3))
        # PSUM: score/proj/outproj pool 3 banks x2 bufs, av 1 bank x2 = 8
        sp = ctx.enter_context(tc.tile_pool(name="sp", bufs=2, space="PSUM"))
        avp = ctx.enter_context(tc.tile_pool(name="avp", bufs=2, space="PSUM"))
        dramp = ctx.enter_context(tc.tile_pool(name="dramp", bufs=2, space="DRAM"))

        # ---- constants to SBUF ----
        w_sb = consts.tile([128, 3 * CCH * 128], BF16, tag="w")
        for s in range(3):
            for j in range(CCH):
                nc.sync.dma_start(
                    out=w_sb[:, (s * CCH + j) * 128:(s * CCH + j + 1) * 128],
                    in_=w_d[s, j * 128:(j + 1) * 128, :],
                )
        b_sb = consts.tile([128, 3], F32, tag="b")
        nc.sync.dma_start(out=b_sb[:], in_=b_d[:, :])
        wo_sb = consts.tile([128, D_MODEL], BF16, tag="wo")
        nc.sync.dma_start(out=wo_sb[:], in_=wo_d[:, :])
        mask_sb = consts.tile([128, NPAT * QT], BF16, tag="mask")
        for p in range(NPAT):
            nc.sync.dma_start(out=mask_sb[:, p * QT:(p + 1) * QT], in_=mk_d[p, :, :])
        id_sb = consts.tile([128, 128], BF16, tag="id")
        nc.sync.dma_start(out=id_sb[:], in_=id_d[:, :])

        # ---- x input, chunked per (tok tile, contraction chunk) ----
        xT_sb = []
        for j in range(CCH):
            t = big.tile([128, T], BF16, tag=f"xT{j}")
            xT_sb.append(t)
        for tt in range(NTOKT):
            for j in range(CCH):
                nc.sync.dma_start(
                    out=xT_sb[j][:, tt * TOKT:(tt + 1) * TOKT],
                    in_=xT_d[j * 128:(j + 1) * 128, tt * TOKT:(tt + 1) * TOKT],
                )

        # ---- projections: one merged (m=128) matmul chain per (s, tt) ----
        QT_sb = big.tile([128, T], BF16, tag="Q")
        KT_sb = big.tile([128, T], BF16, tag="K")
        VT_sb = big.tile([128, T], BF16, tag="VT")
        dests = [QT_sb, KT_sb, VT_sb]
        for tt in range(NTOKT):
            for s in range(3):
                pp = sp.tile([128, TOKT], F32, tag="sc")
                for j in range(CCH):
                    base = (s * CCH + j) * 128
                    nc.tensor.matmul(
                        pp[:], w_sb[:, base:base + 128],
                        xT_sb[j][:, tt * TOKT:(tt + 1) * TOKT],
                        start=(j == 0), stop=(j == CCH - 1),
                    )
                nc.vector.tensor_scalar_add(
                    dests[s][:, tt * TOKT:(tt + 1) * TOKT],
                    pp[:], b_sb[:, s:s + 1],
                )

        # ---- V2 per 128-token key chunk, stride 256 cols:
        #   [0:64]=V_A  [64:192]=ones  [192:256]=V_B
        # lhsT A = cols 0:128   -> psum rows 0:64 AV_A, 64:128 denom_A (x64)
        # lhsT B = cols 128:256 -> psum rows 0:64 denom_B (x64), 64:128 AV_B
        V_sb = big.tile([128, (T // 128) * VST], BF16, tag="V")
        v3 = V_sb[:].rearrange("p (t c) -> p t c", c=VST)
        nc.vector.memset(v3[:, :, 64:192], 1.0)
        for tt4 in range(T // 128):
            tp = sp.tile([128, 128], BF16, tag="sc")
            nc.tensor.transpose(tp[:], VT_sb[:, tt4 * 128:(tt4 + 1) * 128], id_sb[:])
            # single strided copy: tp cols 0:64 -> V2 cols 0:64 (V_A),
            # tp cols 64:128 -> V2 cols 192:256 (V_B)
            src = tp[:]
            dst = V_sb[:, tt4 * VST:(tt4 + 1) * VST]
            nc.vector.tensor_copy(
                bass.AP(tensor=dst.tensor, offset=dst.offset,
                        ap=[dst.ap[0], [192, 2], [1, 64]]),
                bass.AP(tensor=src.tensor, offset=src.offset,
                        ap=[src.ap[0], [64, 2], [1, 64]]),
            )

        # ---- attention + out-projection, software-pipelined ----
        allgroups = []
        for qi in range(NQT):
            nsteps = (qi + 1) * QT // KC
            jobs = [(kc, h) for kc in range(nsteps) for h in (0, 1)]
            for g0 in range(0, len(jobs), GRP):
                allgroups.append(
                    (qi, jobs[g0:g0 + GRP], g0 == 0, g0 + GRP >= len(jobs)))

        av_tiles = {}

        def issue_scores(G):
            qi, grp, first, last = G
            qs = qi * QT
            sc = sp.tile([128, GRP * QT], F32, tag="sc")
            for ji, (kc, h) in enumerate(grp):
                nc.tensor.matmul(
                    sc[:, ji * QT:(ji + 1) * QT],
                    KT_sb[64 * h:64 * h + 64, kc * KC:(kc + 1) * KC],
                    QT_sb[64 * h:64 * h + 64, qs:qs + QT],
                    start=True, stop=True,
                )
            return sc

        def issue_rest(G, sc):
            qi, grp, first, last = G
            qs = qi * QT
            nsteps = (qi + 1) * QT // KC
            if first:
                av_tiles[qi] = avp.tile([128, 2 * QT], F32, tag="av", name="av")
            av = av_tiles[qi]
            width = len(grp) * QT
            pt = ptp.tile([128, GRP * QT], BF16, tag="pt")
            nc.scalar.activation(
                pt[:, :width], sc[:, :width], AF.Exp,
                scale=1.0 / math.sqrt(HEAD_DIM),
            )
            for ji, (kc, h) in enumerate(grp):
                ptj = pt[:, ji * QT:(ji + 1) * QT]
                if kc >= nsteps - NPAT:  # diagonal straddle
                    pat = kc - (nsteps - NPAT)
                    m = mask_sb[:, pat * QT:(pat + 1) * QT]
                    nc.vector.tensor_mul(ptj, ptj, m)
                nc.tensor.matmul(
                    av[:, h * QT:(h + 1) * QT],
                    V_sb[:, kc * VST + 128 * h:kc * VST + 128 * h + 128],
                    ptj, start=(kc == 0), stop=(kc == nsteps - 1),
                )
            if not last:
                return
            # normalize: reciprocal of the 64x-replicated denominator blocks
            av = av_tiles.pop(qi)
            r = rp.tile([128, QT], F32, tag="r")
            nc.vector.reciprocal(r[64:128, :], av[64:128, 0:QT])
            nc.vector.reciprocal(r[0:64, :], av[0:64, QT:2 * QT])
            # partition-broadcast via DRAM bounce (stride-0 partition reads
            # are only legal from DRAM)
            rd = dramp.tile([1, 2 * QT], F32, tag="rd")
            nc.sync.dma_start(out=rd[0:1, 0:QT], in_=r[64:65, 0:QT])
            nc.sync.dma_start(out=rd[0:1, QT:2 * QT], in_=r[0:1, 0:QT])
            rbc = rp.tile([128, QT], F32, tag="rbc")
            rdA = rd[0:1, 0:QT]
            rdB = rd[0:1, QT:2 * QT]
            nc.gpsimd.dma_start(
                out=rbc[0:64, :],
                in_=bass.AP(tensor=rdA.tensor, offset=rdA.offset,
                            ap=[[0, 64]] + list(rdA.ap[1:])))
            nc.gpsimd.dma_start(
                out=rbc[64:128, :],
                in_=bass.AP(tensor=rdB.tensor, offset=rdB.offset,
                            ap=[[0, 64]] + list(rdB.ap[1:])))
            # fused normalize + PSUM->SBUF copy
            hv = hvp.tile([128, QT], BF16, tag="hv")
            nc.vector.tensor_mul(hv[0:64, :], av[0:64, 0:QT], rbc[0:64, :])
            nc.vector.tensor_mul(
                hv[64:128, :], av[64:128, QT:2 * QT], rbc[64:128, :])
            # merged out projection for this query tile: outT[dout, q]
            for dch in range(CCH):
                op = sp.tile([128, QT], F32, tag="sc")
                nc.tensor.matmul(
                    op[:], wo_sb[:, dch * 128:(dch + 1) * 128], hv[:],
                    start=True, stop=True,
                )
                ot = osb.tile([128, QT], BF16, tag="ot")
                nc.vector.tensor_copy(ot[:], op[:])
                nc.sync.dma_start(
                    out=outT_d[dch * 128:(dch + 1) * 128, qs:qs + QT], in_=ot[:],
                )

        prev = None
        for G in allgroups:
            sc = issue_scores(G)
            if prev is not None:
                issue_rest(*prev)
            prev = (G, sc)
        issue_rest(*prev)
    nc.finalize()
    return nc


def _host_inputs(x, wq, bq, wk, bk, wv, bv, wo):
    """Per-core input maps. Slot A of core c = head c; slot B = head 8+c
    (cores 0-3) or a dummy zero head (cores 4-7)."""
    bf16 = ml_dtypes.bfloat16
    xT = np.ascontiguousarray(x[0].T).astype(bf16)
    masks = np.zeros((NPAT, 128, QT), np.float32)
    dk = np.arange(128)[:, None]
    dq = np.arange(QT)[None, :]
    for p in range(NPAT):
        masks[p] = (dk + 128 * p <= dq)
    masks = masks.astype(bf16)
    ident = np.eye(128, dtype=np.float32).astype(bf16)

    in_maps = []
    for c in range(N_CORES):
        hA = c
        hB = 8 + c if c < 4 else None
        w = np.zeros((3, D_MODEL, 128), np.float32)
        b = np.zeros((128, 3), np.float32)
        wo2 = np.zeros((128, D_MODEL), np.float32)
        for s, (W, B) in enumerate(((wq, bq), (wk, bk), (wv, bv))):
            w[s, :, 0:64] = W[hA]
            b[0:64, s] = B[hA]
            if hB is not None:
                w[s, :, 64:128] = W[hB]
                b[64:128, s] = B[hB]
        wo2[0:64, :] = wo[hA * 64:(hA + 1) * 64, :]
        if hB is not None:
            wo2[64:128, :] = wo[hB * 64:(hB + 1) * 64, :]
        in_maps.append({
            "xT": xT,
            "wqkv": w.astype(bf16),
            "bqkv": b.astype(np.float32),
            "wo2": wo2.astype(bf16),
            "masks": masks,
            "ident": ident,
        })
    return in_maps


def kernel(_trace=False, _tmpdir=None, **inputs):
    x = np.asarray(inputs["x"], np.float32)
    args = (x,
            np.asarray(inputs["wq"], np.float32), np.asarray(inputs["bq"], np.float32),
            np.asarray(inputs["wk"], np.float32), np.asarray(inputs["bk"], np.float32),
            np.asarray(inputs["wv"], np.float32), np.asarray(inputs["bv"], np.float32),
            np.asarray(inputs["wo"], np.float32))
    bo = np.asarray(inputs["bo"], np.float32)

    if "nc" not in _PROGRAM_CACHE:
        _PROGRAM_CACHE["nc"] = build_program()
    nc = _PROGRAM_CACHE["nc"]

    in_maps = _host_inputs(*args)
    res = run_bass_kernel_spmd(
        nc, in_maps, list(range(N_CORES)), trace=_trace, tmpdir=_tmpdir,
    )
    acc = np.zeros((D_MODEL, T), np.float32)
    for c in range(N_CORES):
        acc += res.results[c]["outT"].astype(np.float32)
    out = acc.T + bo[None, :]
    if _trace:
        return out[None].astype(np.float32), res
    return out[None].astype(np.float32)


# revision 14
# speedup vs baseline: 1.1419x; 1.1419x over previous
"""Multi-head causal attention (B=1, T=4096, D=768, H=12) on 8 trn2 cores.

Sharding: 16 uniform head-slots (2 per core), 12 real heads + 4 dummy
(zero-weight) slots.  Every core runs the IDENTICAL program (SPMD); cores
differ only in the weight data they receive.  Each core computes, for its
two head-slots, the full causal attention over all 4096 tokens plus that
slot-pair's partial output projection.  The host sums the 8 partial
[768, 4096] bf16 outputs, transposes, and adds the output bias.

v3 layout (per core):
  xT    [768, 4096] bf16   x transposed (host supplies), DMA'd per tok-tile
  QT/KT [128, 4096]  bf16  partitions 0:64 slot A dims, 64:128 slot B
  VT    [128, 4096]  bf16  V pre-transpose, then PE-transposed into
  V2    [128, 32*256] bf16 per key chunk c: [V_A | ones x128 | V_B]
                           -> AV matmul lhsT [V_A|ones64] / [ones64|V_B]
                           gives AV rows plus 64x-replicated denominators
  scores in [128 keys, 256 queries] chunks; 6 chunks per ACT exp group;
  av PSUM [128, 512] = ONE bank (A cols 0:256, B cols 256:512), double
  buffered so query tiles overlap.  Normalize: reciprocal_approx_fast of
  the replicated denom block, DRAM-bounce partition broadcast, division
  fused into the PSUM->SBUF copy, merged (k=128) out-proj, bf16 output.
  The issue order is software-pipelined: scores of group i+1 are issued
  before exp/AV of group i so the PE never sits behind the ACT exp.
"""

import math
import numpy as np
import ml_dtypes
from contextlib import ExitStack

import concourse.bass as bass
import concourse.bacc as bacc
import concourse.mybir as mybir
import concourse.tile as tile
from concourse.bass_utils import run_bass_kernel_spmd

BF16 = mybir.dt.bfloat16
F32 = mybir.dt.float32
AF = mybir.ActivationFunctionType

T = 4096
D_MODEL = 768
HEAD_DIM = 64
N_HEADS = 12
N_CORES = 8
QT = 512                  # query tile width
KC = 128                  # key chunk (psum partition dim)
GRP = 2                   # score chunk-jobs per exp group -> ACT free dim 1024
NPAT = 4                  # straddle mask patterns per query tile (QT//KC)
NQT = T // QT             # 16 query tiles
CCH = D_MODEL // 128      # 6 contraction chunks
TOKT = 512                # token tile for projections
NTOKT = T // TOKT
VST = 256                 # V2 stride per 128-key chunk

_PROGRAM_CACHE = {}


def build_program():
    nc = bacc.Bacc(None)

    xT_d = nc.declare_dram_parameter("xT", [D_MODEL, T], BF16, isOutput=False)
    w_d = nc.declare_dram_parameter("wqkv", [3, D_MODEL, 128], BF16, isOutput=False)
    b_d = nc.declare_dram_parameter("bqkv", [128, 3], F32, isOutput=False)
    wo_d = nc.declare_dram_parameter("wo2", [128, D_MODEL], BF16, isOutput=False)
    mk_d = nc.declare_dram_parameter("masks", [NPAT, 128, QT], BF16, isOutput=False)
    id_d = nc.declare_dram_parameter("ident", [128, 128], BF16, isOutput=False)
    outA_d = nc.declare_dram_parameter("outA", [D_MODEL, T], BF16, isOutput=True)
    outB_d = nc.declare_dram_parameter("outB", [D_MODEL, T], BF16, isOutput=True)
    denA_d = nc.declare_dram_parameter("denA", [1, T], BF16, isOutput=True)
    denB_d = nc.declare_dram_parameter("denB", [1, T], BF16, isOutput=True)

    with tile.TileContext(nc) as tc, ExitStack() as ctx:
        consts = ctx.enter_context(tc.tile_pool(name="consts", bufs=1))
        big = ctx.enter_context(tc.tile_pool(name="big", bufs=1))
        ptp = ctx.enter_context(tc.tile_pool(name="ptp", bufs=3))
        rp = ctx.enter_context(tc.tile_pool(name="rp", bufs=2))
        hvp = ctx.enter_context(tc.tile_pool(name="hvp", bufs=2))
        osb = ctx.enter_context(tc.tile_pool(name="osb", bufs=3))
        # PSUM: score/proj/outproj pool 3 banks x2 bufs, av 1 bank x2 = 8
        sp = ctx.enter_context(tc.tile_pool(name="sp", bufs=2, space="PSUM"))
        avp = ctx.enter_context(tc.tile_pool(name="avp", bufs=2, space="PSUM"))
        dramp = ctx.enter_context(tc.tile_pool(name="dramp", bufs=2, space="DRAM"))

        # ---- constants to SBUF ----
        w_sb = consts.tile([128, 3 * CCH * 128], BF16, tag="w")
        for s in range(3):
            for j in range(CCH):
                nc.sync.dma_start(
                    out=w_sb[:, (s * CCH + j) * 128:(s * CCH + j + 1) * 128],
                    in_=w_d[s, j * 128:(j + 1) * 128, :],
                )
        b_sb = consts.tile([128, 3], F32, tag="b")
        nc.sync.dma_start(out=b_sb[:], in_=b_d[:, :])
        wo_sb = consts.tile([128, D_MODEL], BF16, tag="wo")
        nc.sync.dma_start(out=wo_sb[:], in_=wo_d[:, :])
        mask_sb = consts.tile([128, NPAT * QT], BF16, tag="mask")
        for p in range(NPAT):
            nc.sync.dma_start(out=mask_sb[:, p * QT:(p + 1) * QT], in_=mk_d[p, :, :])
        id_sb = consts.tile([128, 128], BF16, tag="id")
        nc.sync.dma_start(out=id_sb[:], in_=id_d[:, :])

        # ---- x input, chunked per (tok tile, contraction chunk) ----
        xT_sb = []
        for j in range(CCH):
            t = big.tile([128, T], BF16, tag=f"xT{j}")
            xT_sb.append(t)
        for tt in range(NTOKT):
            for j in range(CCH):
                nc.sync.dma_start(
                    out=xT_sb[j][:, tt * TOKT:(tt + 1) * TOKT],
                    in_=xT_d[j * 128:(j + 1) * 128, tt * TOKT:(tt + 1) * TOKT],
                )

        # ---- projections: one merged (m=128) matmul chain per (s, tt) ----
        QT_sb = big.tile([128, T], BF16, tag="Q")
        KT_sb = big.tile([128, T], BF16, tag="K")
        VT_sb = big.tile([128, T], BF16, tag="VT")
        dests = [QT_sb, KT_sb, VT_sb]
        for tt in range(NTOKT):
            for s in range(3):
                pp = sp.tile([128, TOKT], F32, tag="sc")
                for j in range(CCH):
                    base = (s * CCH + j) * 128
                    nc.tensor.matmul(
                        pp[:], w_sb[:, base:base + 128],
                        xT_sb[j][:, tt * TOKT:(tt + 1) * TOKT],
                        start=(j == 0), stop=(j == CCH - 1),
                    )
                nc.vector.tensor_scalar_add(
                    dests[s][:, tt * TOKT:(tt + 1) * TOKT],
                    pp[:], b_sb[:, s:s + 1],
                )

        # ---- V2 per 128-token key chunk, stride 256 cols:
        #   [0:64]=V_A  [64:192]=ones  [192:256]=V_B
        # lhsT A = cols 0:128   -> psum rows 0:64 AV_A, 64:128 denom_A (x64)
        # lhsT B = cols 128:256 -> psum rows 0:64 denom_B (x64), 64:128 AV_B
        V_sb = big.tile([128, (T // 128) * VST], BF16, tag="V")
        v3 = V_sb[:].rearrange("p (t c) -> p t c", c=VST)
        nc.vector.memset(v3[:, :, 64:192], 1.0)
        for tt4 in range(T // 128):
            tp = sp.tile([128, 128], BF16, tag="sc")
            nc.tensor.transpose(tp[:], VT_sb[:, tt4 * 128:(tt4 + 1) * 128], id_sb[:])
            # single strided copy: tp cols 0:64 -> V2 cols 0:64 (V_A),
            # tp cols 64:128 -> V2 cols 192:256 (V_B)
            src = tp[:]
            dst = V_sb[:, tt4 * VST:(tt4 + 1) * VST]
            nc.vector.tensor_copy(
                bass.AP(tensor=dst.tensor, offset=dst.offset,
                        ap=[dst.ap[0], [192, 2], [1, 64]]),
                bass.AP(tensor=src.tensor, offset=src.offset,
                        ap=[src.ap[0], [64, 2], [1, 64]]),
            )

        # ---- attention + out-projection, software-pipelined ----
        allgroups = []
        for qi in range(NQT):
            nsteps = (qi + 1) * QT // KC
            jobs = [(kc, h) for kc in range(nsteps) for h in (0, 1)]
            for g0 in range(0, len(jobs), GRP):
                allgroups.append(
                    (qi, jobs[g0:g0 + GRP], g0 == 0, g0 + GRP >= len(jobs)))

        av_tiles = {}

        def issue_scores(G):
            qi, grp, first, last = G
            qs = qi * QT
            sc = sp.tile([128, GRP * QT], F32, tag="sc")
            for ji, (kc, h) in enumerate(grp):
                nc.tensor.matmul(
                    sc[:, ji * QT:(ji + 1) * QT],
                    KT_sb[64 * h:64 * h + 64, kc * KC:(kc + 1) * KC],
                    QT_sb[64 * h:64 * h + 64, qs:qs + QT],
                    start=True, stop=True,
                )
            return sc

        def issue_rest(G, sc):
            qi, grp, first, last = G
            qs = qi * QT
            nsteps = (qi + 1) * QT // KC
            if first:
                av_tiles[qi] = avp.tile([128, 2 * QT], F32, tag="av", name="av")
            av = av_tiles[qi]
            width = len(grp) * QT
            pt = ptp.tile([128, GRP * QT], BF16, tag="pt")
            nc.scalar.activation(
                pt[:, :width], sc[:, :width], AF.Exp,
                scale=1.0 / math.sqrt(HEAD_DIM),
            )
            for ji, (kc, h) in enumerate(grp):
                ptj = pt[:, ji * QT:(ji + 1) * QT]
                if kc >= nsteps - NPAT:  # diagonal straddle
                    pat = kc - (nsteps - NPAT)
                    m = mask_sb[:, pat * QT:(pat + 1) * QT]
                    nc.vector.tensor_mul(ptj, ptj, m)
                nc.tensor.matmul(
                    av[:, h * QT:(h + 1) * QT],
                    V_sb[:, kc * VST + 128 * h:kc * VST + 128 * h + 128],
                    ptj, start=(kc == 0), stop=(kc == nsteps - 1),
                )
            if not last:
                return
            # unnormalized per-slot out-projection; host divides by the
            # denominators (flash-attention-style partial combination).
            av = av_tiles.pop(qi)
            hvA = hvp.tile([128, QT], BF16, tag="hvA", name="hvA")
            nc.vector.tensor_copy(hvA[:], av[:, 0:QT])
            hvB = hvp.tile([128, QT], BF16, tag="hvB", name="hvB")
            nc.vector.tensor_copy(hvB[:], av[:, QT:2 * QT])
            nc.sync.dma_start(out=denA_d[0:1, qs:qs + QT], in_=hvA[64:65, :])
            nc.sync.dma_start(out=denB_d[0:1, qs:qs + QT], in_=hvB[0:1, :])
            for dch in range(CCH):
                op = sp.tile([128, QT], F32, tag="sc")
                nc.tensor.matmul(
                    op[:], wo_sb[0:64, dch * 128:(dch + 1) * 128], hvA[0:64, :],
                    start=True, stop=True,
                )
                ot = osb.tile([128, QT], BF16, tag="ot")
                nc.vector.tensor_copy(ot[:], op[:])
                nc.sync.dma_start(
                    out=outA_d[dch * 128:(dch + 1) * 128, qs:qs + QT], in_=ot[:],
                )
                op2 = sp.tile([128, QT], F32, tag="sc", name="op2")
                nc.tensor.matmul(
                    op2[:], wo_sb[64:128, dch * 128:(dch + 1) * 128],
                    hvB[64:128, :], start=True, stop=True,
                )
                ot2 = osb.tile([128, QT], BF16, tag="ot", name="ot2")
                nc.vector.tensor_copy(ot2[:], op2[:])
                nc.sync.dma_start(
                    out=outB_d[dch * 128:(dch + 1) * 128, qs:qs + QT], in_=ot2[:],
                )

        prev = None
        for G in allgroups:
            sc = issue_scores(G)
            if prev is not None:
                issue_rest(*prev)
            prev = (G, sc)
        issue_rest(*prev)
    nc.finalize()
    return nc


def _host_inputs(x, wq, bq, wk, bk, wv, bv, wo):
    """Per-core input maps. Slot A of core c = head c; slot B = head 8+c
    (cores 0-3) or a dummy zero head (cores 4-7)."""
    bf16 = ml_dtypes.bfloat16
    xT = np.ascontiguousarray(x[0].T).astype(bf16)
    masks = np.zeros((NPAT, 128, QT), np.float32)
    dk = np.arange(128)[:, None]
    dq = np.arange(QT)[None, :]
    for p in range(NPAT):
        masks[p] = (dk + 128 * p <= dq)
    masks = masks.astype(bf16)
    ident = np.eye(128, dtype=np.float32).astype(bf16)

    in_maps = []
    for c in range(N_CORES):
        hA = c
        hB = 8 + c if c < 4 else None
        w = np.zeros((3, D_MODEL, 128), np.float32)
        b = np.zeros((128, 3), np.float32)
        wo2 = np.zeros((128, D_MODEL), np.float32)
        for s, (W, B) in enumerate(((wq, bq), (wk, bk), (wv, bv))):
            w[s, :, 0:64] = W[hA]
            b[0:64, s] = B[hA]
            if hB is not None:
                w[s, :, 64:128] = W[hB]
                b[64:128, s] = B[hB]
        wo2[0:64, :] = wo[hA * 64:(hA + 1) * 64, :]
        if hB is not None:
            wo2[64:128, :] = wo[hB * 64:(hB + 1) * 64, :]
        in_maps.append({
            "xT": xT,
            "wqkv": w.astype(bf16),
            "bqkv": b.astype(np.float32),
            "wo2": wo2.astype(bf16),
            "masks": masks,
            "ident": ident,
        })
    return in_maps


def kernel(_trace=False, _tmpdir=None, **inputs):
    x = np.asarray(inputs["x"], np.float32)
    args = (x,
            np.asarray(inputs["wq"], np.float32), np.asarray(inputs["bq"], np.float32),
            np.asarray(inputs["wk"], np.float32), np.asarray(inputs["bk"], np.float32),
            np.asarray(inputs["wv"], np.float32), np.asarray(inputs["bv"], np.float32),
            np.asarray(inputs["wo"], np.float32))
    bo = np.asarray(inputs["bo"], np.float32)

    if "nc" not in _PROGRAM_CACHE:
        _PROGRAM_CACHE["nc"] = build_program()
    nc = _PROGRAM_CACHE["nc"]

    in_maps = _host_inputs(*args)
    res = run_bass_kernel_spmd(
        nc, in_maps, list(range(N_CORES)), trace=_trace, tmpdir=_tmpdir,
    )
    acc = np.zeros((D_MODEL, T), np.float32)
    for c in range(N_CORES):
        r = res.results[c]
        acc += r["outA"].astype(np.float32) / r["denA"].astype(np.float32)
        acc += r["outB"].astype(np.float32) / r["denB"].astype(np.float32)
    out = acc.T + bo[None, :]
    if _trace:
        return out[None].astype(np.float32), res
    return out[None].astype(np.float32)


# revision 15
# speedup vs baseline: 1.2298x; 1.0770x over previous
"""Multi-head causal attention (B=1, T=4096, D=768, H=12) on 8 trn2 cores.

Sharding: 16 uniform head-slots (2 per core), 12 real heads + 4 dummy
(zero-weight) slots.  Every core runs the IDENTICAL program (SPMD); cores
differ only in the weight data they receive.  Each core computes, for its
two head-slots, the full causal attention over all 4096 tokens plus that
slot-pair's partial output projection.  The host sums the 8 partial
[768, 4096] bf16 outputs, transposes, and adds the output bias.

v3 layout (per core):
  xT    [768, 4096] bf16   x transposed (host supplies), DMA'd per tok-tile
  QT/KT [128, 4096]  bf16  partitions 0:64 slot A dims, 64:128 slot B
  VT    [128, 4096]  bf16  V pre-transpose, then PE-transposed into
  V2    [128, 32*256] bf16 per key chunk c: [V_A | ones x128 | V_B]
                           -> AV matmul lhsT [V_A|ones64] / [ones64|V_B]
                           gives AV rows plus 64x-replicated denominators
  scores in [128 keys, 256 queries] chunks; 6 chunks per ACT exp group;
  av PSUM [128, 512] = ONE bank (A cols 0:256, B cols 256:512), double
  buffered so query tiles overlap.  Normalize: reciprocal_approx_fast of
  the replicated denom block, DRAM-bounce partition broadcast, division
  fused into the PSUM->SBUF copy, merged (k=128) out-proj, bf16 output.
  The issue order is software-pipelined: scores of group i+1 are issued
  before exp/AV of group i so the PE never sits behind the ACT exp.
"""

import math
import numpy as np
import ml_dtypes
from contextlib import ExitStack

import concourse.bass as bass
import concourse.bacc as bacc
import concourse.mybir as mybir
import concourse.tile as tile
from concourse.bass_utils import run_bass_kernel_spmd

BF16 = mybir.dt.bfloat16
F32 = mybir.dt.float32
AF = mybir.ActivationFunctionType

T = 4096
D_MODEL = 768
HEAD_DIM = 64
N_HEADS = 12
N_CORES = 8
QT = 512                  # query tile width
KC = 128                  # key chunk (psum partition dim)
GRP = 2                   # score chunk-jobs per exp group -> ACT free dim 1024
NPAT = 4                  # straddle mask patterns per query tile (QT//KC)
NQT = T // QT             # 16 query tiles
CCH = D_MODEL // 128      # 6 contraction chunks
TOKT = 512                # token tile for projections
NTOKT = T // TOKT
VST = 256                 # V2 stride per 128-key chunk

_PROGRAM_CACHE = {}


def build_program():
    nc = bacc.Bacc(None)

    xT_d = nc.declare_dram_parameter("xT", [D_MODEL, T], BF16, isOutput=False)
    w_d = nc.declare_dram_parameter("wqkv", [3, D_MODEL, 128], BF16, isOutput=False)
    b_d = nc.declare_dram_parameter("bqkv", [128, 3], F32, isOutput=False)
    wo_d = nc.declare_dram_parameter("wo2", [128, D_MODEL], BF16, isOutput=False)
    mk_d = nc.declare_dram_parameter("masks", [NPAT, 128, QT], BF16, isOutput=False)
    id_d = nc.declare_dram_parameter("ident", [128, 128], BF16, isOutput=False)
    outA_d = nc.declare_dram_parameter("outA", [D_MODEL, T], BF16, isOutput=True)
    outB_d = nc.declare_dram_parameter("outB", [D_MODEL, T], BF16, isOutput=True)
    denA_d = nc.declare_dram_parameter("denA", [1, T], BF16, isOutput=True)
    denB_d = nc.declare_dram_parameter("denB", [1, T], BF16, isOutput=True)

    with tile.TileContext(nc) as tc, ExitStack() as ctx:
        consts = ctx.enter_context(tc.tile_pool(name="consts", bufs=1))
        big = ctx.enter_context(tc.tile_pool(name="big", bufs=1))
        ptp = ctx.enter_context(tc.tile_pool(name="ptp", bufs=3))
        rp = ctx.enter_context(tc.tile_pool(name="rp", bufs=2))
        hvp = ctx.enter_context(tc.tile_pool(name="hvp", bufs=2))
        osb = ctx.enter_context(tc.tile_pool(name="osb", bufs=3))
        # PSUM: score/proj/outproj pool 3 banks x2 bufs, av 1 bank x2 = 8
        sp = ctx.enter_context(tc.tile_pool(name="sp", bufs=3, space="PSUM"))
        avp = ctx.enter_context(tc.tile_pool(name="avp", bufs=1, space="PSUM"))
        dramp = ctx.enter_context(tc.tile_pool(name="dramp", bufs=2, space="DRAM"))

        # ---- constants to SBUF ----
        w_sb = consts.tile([128, 3 * CCH * 128], BF16, tag="w")
        for s in range(3):
            for j in range(CCH):
                nc.sync.dma_start(
                    out=w_sb[:, (s * CCH + j) * 128:(s * CCH + j + 1) * 128],
                    in_=w_d[s, j * 128:(j + 1) * 128, :],
                )
        b_sb = consts.tile([128, 3], F32, tag="b")
        nc.sync.dma_start(out=b_sb[:], in_=b_d[:, :])
        wo_sb = consts.tile([128, D_MODEL], BF16, tag="wo")
        nc.sync.dma_start(out=wo_sb[:], in_=wo_d[:, :])
        mask_sb = consts.tile([128, NPAT * QT], BF16, tag="mask")
        for p in range(NPAT):
            nc.sync.dma_start(out=mask_sb[:, p * QT:(p + 1) * QT], in_=mk_d[p, :, :])
        id_sb = consts.tile([128, 128], BF16, tag="id")
        nc.sync.dma_start(out=id_sb[:], in_=id_d[:, :])

        # ---- x input, chunked per (tok tile, contraction chunk) ----
        xT_sb = []
        for j in range(CCH):
            t = big.tile([128, T], BF16, tag=f"xT{j}")
            xT_sb.append(t)
        for tt in range(NTOKT):
            for j in range(CCH):
                nc.sync.dma_start(
                    out=xT_sb[j][:, tt * TOKT:(tt + 1) * TOKT],
                    in_=xT_d[j * 128:(j + 1) * 128, tt * TOKT:(tt + 1) * TOKT],
                )

        # ---- projections: one merged (m=128) matmul chain per (s, tt) ----
        QT_sb = big.tile([128, T], BF16, tag="Q")
        KT_sb = big.tile([128, T], BF16, tag="K")
        VT_sb = big.tile([128, T], BF16, tag="VT")
        dests = [QT_sb, KT_sb, VT_sb]
        for tt in range(NTOKT):
            for s in range(3):
                pp = sp.tile([128, TOKT], F32, tag="sc")
                for j in range(CCH):
                    base = (s * CCH + j) * 128
                    nc.tensor.matmul(
                        pp[:], w_sb[:, base:base + 128],
                        xT_sb[j][:, tt * TOKT:(tt + 1) * TOKT],
                        start=(j == 0), stop=(j == CCH - 1),
                    )
                nc.vector.tensor_scalar_add(
                    dests[s][:, tt * TOKT:(tt + 1) * TOKT],
                    pp[:], b_sb[:, s:s + 1],
                )

        # ---- V2 per 128-token key chunk, stride 256 cols:
        #   [0:64]=V_A  [64:192]=ones  [192:256]=V_B
        # lhsT A = cols 0:128   -> psum rows 0:64 AV_A, 64:128 denom_A (x64)
        # lhsT B = cols 128:256 -> psum rows 0:64 denom_B (x64), 64:128 AV_B
        V_sb = big.tile([128, (T // 128) * VST], BF16, tag="V")
        v3 = V_sb[:].rearrange("p (t c) -> p t c", c=VST)
        nc.vector.memset(v3[:, :, 64:192], 1.0)
        for tt4 in range(T // 128):
            tp = sp.tile([128, 128], BF16, tag="sc")
            nc.tensor.transpose(tp[:], VT_sb[:, tt4 * 128:(tt4 + 1) * 128], id_sb[:])
            # single strided copy: tp cols 0:64 -> V2 cols 0:64 (V_A),
            # tp cols 64:128 -> V2 cols 192:256 (V_B)
            src = tp[:]
            dst = V_sb[:, tt4 * VST:(tt4 + 1) * VST]
            nc.vector.tensor_copy(
                bass.AP(tensor=dst.tensor, offset=dst.offset,
                        ap=[dst.ap[0], [192, 2], [1, 64]]),
                bass.AP(tensor=src.tensor, offset=src.offset,
                        ap=[src.ap[0], [64, 2], [1, 64]]),
            )

        # ---- attention + out-projection, software-pipelined ----
        allgroups = []
        for qi in range(NQT):
            nsteps = (qi + 1) * QT // KC
            jobs = [(kc, h) for kc in range(nsteps) for h in (0, 1)]
            for g0 in range(0, len(jobs), GRP):
                allgroups.append(
                    (qi, jobs[g0:g0 + GRP], g0 == 0, g0 + GRP >= len(jobs)))

        av_tiles = {}

        def issue_scores(G):
            qi, grp, first, last = G
            qs = qi * QT
            sc = sp.tile([128, GRP * QT], F32, tag="sc")
            for ji, (kc, h) in enumerate(grp):
                nc.tensor.matmul(
                    sc[:, ji * QT:(ji + 1) * QT],
                    KT_sb[64 * h:64 * h + 64, kc * KC:(kc + 1) * KC],
                    QT_sb[64 * h:64 * h + 64, qs:qs + QT],
                    start=True, stop=True,
                )
            return sc

        def issue_rest(G, sc):
            qi, grp, first, last = G
            qs = qi * QT
            nsteps = (qi + 1) * QT // KC
            if first:
                av_tiles[qi] = avp.tile([128, 2 * QT], F32, tag="av", name="av")
            av = av_tiles[qi]
            width = len(grp) * QT
            pt = ptp.tile([128, GRP * QT], BF16, tag="pt")
            nc.scalar.activation(
                pt[:, :width], sc[:, :width], AF.Exp,
                scale=1.0 / math.sqrt(HEAD_DIM),
            )
            for ji, (kc, h) in enumerate(grp):
                ptj = pt[:, ji * QT:(ji + 1) * QT]
                if kc >= nsteps - NPAT:  # diagonal straddle
                    pat = kc - (nsteps - NPAT)
                    m = mask_sb[:, pat * QT:(pat + 1) * QT]
                    nc.vector.tensor_mul(ptj, ptj, m)
                nc.tensor.matmul(
                    av[:, h * QT:(h + 1) * QT],
                    V_sb[:, kc * VST + 128 * h:kc * VST + 128 * h + 128],
                    ptj, start=(kc == 0), stop=(kc == nsteps - 1),
                )
            if not last:
                return
            # unnormalized per-slot out-projection; host divides by the
            # denominators (flash-attention-style partial combination).
            av = av_tiles.pop(qi)
            hvA = hvp.tile([128, QT], BF16, tag="hvA", name="hvA")
            nc.vector.tensor_copy(hvA[:], av[:, 0:QT])
            hvB = hvp.tile([128, QT], BF16, tag="hvB", name="hvB")
            nc.vector.tensor_copy(hvB[:], av[:, QT:2 * QT])
            nc.sync.dma_start(out=denA_d[0:1, qs:qs + QT], in_=hvA[64:65, :])
            nc.sync.dma_start(out=denB_d[0:1, qs:qs + QT], in_=hvB[0:1, :])
            for dch in range(CCH):
                op = sp.tile([128, QT], F32, tag="sc")
                nc.tensor.matmul(
                    op[:], wo_sb[0:64, dch * 128:(dch + 1) * 128], hvA[0:64, :],
                    start=True, stop=True,
                )
                ot = osb.tile([128, QT], BF16, tag="ot")
                nc.vector.tensor_copy(ot[:], op[:])
                nc.sync.dma_start(
                    out=outA_d[dch * 128:(dch + 1) * 128, qs:qs + QT], in_=ot[:],
                )
                op2 = sp.tile([128, QT], F32, tag="sc", name="op2")
                nc.tensor.matmul(
                    op2[:], wo_sb[64:128, dch * 128:(dch + 1) * 128],
                    hvB[64:128, :], start=True, stop=True,
                )
                ot2 = osb.tile([128, QT], BF16, tag="ot", name="ot2")
                nc.vector.tensor_copy(ot2[:], op2[:])
                nc.sync.dma_start(
                    out=outB_d[dch * 128:(dch + 1) * 128, qs:qs + QT], in_=ot2[:],
                )

        from collections import deque
        pend = deque()
        for G in allgroups:
            sc = issue_scores(G)
            pend.append((G, sc))
            if len(pend) > 2:
                issue_rest(*pend.popleft())
        while pend:
            issue_rest(*pend.popleft())
    nc.finalize()
    return nc


def _host_inputs(x, wq, bq, wk, bk, wv, bv, wo):
    """Per-core input maps. Slot A of core c = head c; slot B = head 8+c
    (cores 0-3) or a dummy zero head (cores 4-7)."""
    bf16 = ml_dtypes.bfloat16
    xT = np.ascontiguousarray(x[0].T).astype(bf16)
    masks = np.zeros((NPAT, 128, QT), np.float32)
    dk = np.arange(128)[:, None]
    dq = np.arange(QT)[None, :]
    for p in range(NPAT):
        masks[p] = (dk + 128 * p <= dq)
    masks = masks.astype(bf16)
    ident = np.eye(128, dtype=np.float32).astype(bf16)

    in_maps = []
    for c in range(N_CORES):
        hA = c
        hB = 8 + c if c < 4 else None
        w = np.zeros((3, D_MODEL, 128), np.float32)
        b = np.zeros((128, 3), np.float32)
        wo2 = np.zeros((128, D_MODEL), np.float32)
        for s, (W, B) in enumerate(((wq, bq), (wk, bk), (wv, bv))):
            w[s, :, 0:64] = W[hA]
            b[0:64, s] = B[hA]
            if hB is not None:
                w[s, :, 64:128] = W[hB]
                b[64:128, s] = B[hB]
        wo2[0:64, :] = wo[hA * 64:(hA + 1) * 64, :]
        if hB is not None:
            wo2[64:128, :] = wo[hB * 64:(hB + 1) * 64, :]
        in_maps.append({
            "xT": xT,
            "wqkv": w.astype(bf16),
            "bqkv": b.astype(np.float32),
            "wo2": wo2.astype(bf16),
            "masks": masks,
            "ident": ident,
        })
    return in_maps


def kernel(_trace=False, _tmpdir=None, **inputs):
    x = np.asarray(inputs["x"], np.float32)
    args = (x,
            np.asarray(inputs["wq"], np.float32), np.asarray(inputs["bq"], np.float32),
            np.asarray(inputs["wk"], np.float32), np.asarray(inputs["bk"], np.float32),
            np.asarray(inputs["wv"], np.float32), np.asarray(inputs["bv"], np.float32),
            np.asarray(inputs["wo"], np.float32))
    bo = np.asarray(inputs["bo"], np.float32)

    if "nc" not in _PROGRAM_CACHE:
        _PROGRAM_CACHE["nc"] = build_program()
    nc = _PROGRAM_CACHE["nc"]

    in_maps = _host_inputs(*args)
    res = run_bass_kernel_spmd(
        nc, in_maps, list(range(N_CORES)), trace=_trace, tmpdir=_tmpdir,
    )
    acc = np.zeros((D_MODEL, T), np.float32)
    for c in range(N_CORES):
        r = res.results[c]
        acc += r["outA"].astype(np.float32) / r["denA"].astype(np.float32)
        acc += r["outB"].astype(np.float32) / r["denB"].astype(np.float32)
    out = acc.T + bo[None, :]
    if _trace:
        return out[None].astype(np.float32), res
    return out[None].astype(np.float32)


# revision 16
# speedup vs baseline: 1.6188x; 1.3163x over previous
"""Multi-head causal attention (B=1, T=4096, D=768, H=12) on 8 trn2 cores.

Sharding: 16 uniform head-slots (2 per core), 12 real heads + 4 dummy
(zero-weight) slots.  Every core runs the IDENTICAL program (SPMD); cores
differ only in the weight data they receive.  Each core computes, for its
two head-slots, the full causal attention over all 4096 tokens plus that
slot-pair's partial output projection.  The host sums the 8 partial
[768, 4096] bf16 outputs, transposes, and adds the output bias.

v3 layout (per core):
  xT    [768, 4096] bf16   x transposed (host supplies), DMA'd per tok-tile
  QT/KT [128, 4096]  bf16  partitions 0:64 slot A dims, 64:128 slot B
  VT    [128, 4096]  bf16  V pre-transpose, then PE-transposed into
  V2    [128, 32*256] bf16 per key chunk c: [V_A | ones x128 | V_B]
                           -> AV matmul lhsT [V_A|ones64] / [ones64|V_B]
                           gives AV rows plus 64x-replicated denominators
  scores in [128 keys, 256 queries] chunks; 6 chunks per ACT exp group;
  av PSUM [128, 512] = ONE bank (A cols 0:256, B cols 256:512), double
  buffered so query tiles overlap.  Normalize: reciprocal_approx_fast of
  the replicated denom block, DRAM-bounce partition broadcast, division
  fused into the PSUM->SBUF copy, merged (k=128) out-proj, bf16 output.
  The issue order is software-pipelined: scores of group i+1 are issued
  before exp/AV of group i so the PE never sits behind the ACT exp.
"""

import math
import numpy as np
import ml_dtypes
from contextlib import ExitStack

import concourse.bass as bass
import concourse.bacc as bacc
import concourse.mybir as mybir
import concourse.tile as tile
from concourse.bass_utils import run_bass_kernel_spmd

BF16 = mybir.dt.bfloat16
F32 = mybir.dt.float32
AF = mybir.ActivationFunctionType

T = 4096
D_MODEL = 768
HEAD_DIM = 64
N_HEADS = 12
N_CORES = 8
QT = 512                  # query tile width
KC = 128                  # key chunk (psum partition dim)
GRP = 2                   # score chunk-jobs per exp group -> ACT free dim 1024
NPAT = 4                  # straddle mask patterns per query tile (QT//KC)
NQT = T // QT             # 16 query tiles
CCH = D_MODEL // 128      # 6 contraction chunks
TOKT = 512                # token tile for projections
NTOKT = T // TOKT
VST = 256                 # V2 stride per 128-key chunk

_PROGRAM_CACHE = {}


def build_program():
    nc = bacc.Bacc(None)

    xT_d = nc.declare_dram_parameter("xT", [D_MODEL, T], BF16, isOutput=False)
    w_d = nc.declare_dram_parameter("wqkv", [3, D_MODEL, 128], BF16, isOutput=False)
    b_d = nc.declare_dram_parameter("bqkv", [128, 3], F32, isOutput=False)
    wo_d = nc.declare_dram_parameter("wo2", [128, D_MODEL], BF16, isOutput=False)
    mk_d = nc.declare_dram_parameter("masks", [NPAT, 128, QT], BF16, isOutput=False)
    id_d = nc.declare_dram_parameter("ident", [128, 128], BF16, isOutput=False)
    outA_d = nc.declare_dram_parameter("outA", [D_MODEL, T], BF16, isOutput=True)
    outB_d = nc.declare_dram_parameter("outB", [D_MODEL, T], BF16, isOutput=True)
    denA_d = nc.declare_dram_parameter("denA", [1, T], BF16, isOutput=True)
    denB_d = nc.declare_dram_parameter("denB", [1, T], BF16, isOutput=True)

    with tile.TileContext(nc) as tc, ExitStack() as ctx:
        consts = ctx.enter_context(tc.tile_pool(name="consts", bufs=1))
        big = ctx.enter_context(tc.tile_pool(name="big", bufs=1))
        ptp = ctx.enter_context(tc.tile_pool(name="ptp", bufs=4))
        rp = ctx.enter_context(tc.tile_pool(name="rp", bufs=2))
        hvp = ctx.enter_context(tc.tile_pool(name="hvp", bufs=4))
        osb = ctx.enter_context(tc.tile_pool(name="osb", bufs=3))
        # PSUM: score/proj/outproj pool 3 banks x2 bufs, av 1 bank x2 = 8
        sp = ctx.enter_context(tc.tile_pool(name="sp", bufs=3, space="PSUM"))
        avp = ctx.enter_context(tc.tile_pool(name="avp", bufs=1, space="PSUM"))
        dramp = ctx.enter_context(tc.tile_pool(name="dramp", bufs=2, space="DRAM"))

        # ---- constants to SBUF ----
        w_sb = consts.tile([128, 3 * CCH * 128], BF16, tag="w")
        for s in range(3):
            for j in range(CCH):
                nc.sync.dma_start(
                    out=w_sb[:, (s * CCH + j) * 128:(s * CCH + j + 1) * 128],
                    in_=w_d[s, j * 128:(j + 1) * 128, :],
                )
        b_sb = consts.tile([128, 3], F32, tag="b")
        nc.sync.dma_start(out=b_sb[:], in_=b_d[:, :])
        wo_sb = consts.tile([128, D_MODEL], BF16, tag="wo")
        nc.sync.dma_start(out=wo_sb[:], in_=wo_d[:, :])
        mask_sb = consts.tile([128, NPAT * QT], BF16, tag="mask")
        for p in range(NPAT):
            nc.sync.dma_start(out=mask_sb[:, p * QT:(p + 1) * QT], in_=mk_d[p, :, :])
        id_sb = consts.tile([128, 128], BF16, tag="id")
        nc.sync.dma_start(out=id_sb[:], in_=id_d[:, :])

        # ---- x input, chunked per (tok tile, contraction chunk) ----
        xT_sb = []
        for j in range(CCH):
            t = big.tile([128, T], BF16, tag=f"xT{j}")
            xT_sb.append(t)
        for tt in range(NTOKT):
            for j in range(CCH):
                nc.sync.dma_start(
                    out=xT_sb[j][:, tt * TOKT:(tt + 1) * TOKT],
                    in_=xT_d[j * 128:(j + 1) * 128, tt * TOKT:(tt + 1) * TOKT],
                )

        # ---- projections: one merged (m=128) matmul chain per (s, tt) ----
        QT_sb = big.tile([128, T], BF16, tag="Q")
        KT_sb = big.tile([128, T], BF16, tag="K")
        VT_sb = big.tile([128, T], BF16, tag="VT")
        dests = [QT_sb, KT_sb, VT_sb]
        for tt in range(NTOKT):
            for s in range(3):
                pp = sp.tile([128, TOKT], F32, tag="sc")
                for j in range(CCH):
                    base = (s * CCH + j) * 128
                    nc.tensor.matmul(
                        pp[:], w_sb[:, base:base + 128],
                        xT_sb[j][:, tt * TOKT:(tt + 1) * TOKT],
                        start=(j == 0), stop=(j == CCH - 1),
                    )
                nc.vector.tensor_scalar_add(
                    dests[s][:, tt * TOKT:(tt + 1) * TOKT],
                    pp[:], b_sb[:, s:s + 1],
                )

        # ---- V2 per 128-token key chunk, stride 256 cols:
        #   [0:64]=V_A  [64:192]=ones  [192:256]=V_B
        # lhsT A = cols 0:128   -> psum rows 0:64 AV_A, 64:128 denom_A (x64)
        # lhsT B = cols 128:256 -> psum rows 0:64 denom_B (x64), 64:128 AV_B
        V_sb = big.tile([128, (T // 128) * VST], BF16, tag="V")
        v3 = V_sb[:].rearrange("p (t c) -> p t c", c=VST)
        nc.vector.memset(v3[:, :, 64:192], 1.0)
        for tt4 in range(T // 128):
            tp = sp.tile([128, 128], BF16, tag="sc")
            nc.tensor.transpose(tp[:], VT_sb[:, tt4 * 128:(tt4 + 1) * 128], id_sb[:])
            # single strided copy: tp cols 0:64 -> V2 cols 0:64 (V_A),
            # tp cols 64:128 -> V2 cols 192:256 (V_B)
            src = tp[:]
            dst = V_sb[:, tt4 * VST:(tt4 + 1) * VST]
            nc.vector.tensor_copy(
                bass.AP(tensor=dst.tensor, offset=dst.offset,
                        ap=[dst.ap[0], [192, 2], [1, 64]]),
                bass.AP(tensor=src.tensor, offset=src.offset,
                        ap=[src.ap[0], [64, 2], [1, 64]]),
            )

        # ---- attention + out-projection, software-pipelined ----
        allgroups = []
        for qi in range(NQT):
            nsteps = (qi + 1) * QT // KC
            jobs = [(kc, h) for kc in range(nsteps) for h in (0, 1)]
            for g0 in range(0, len(jobs), GRP):
                allgroups.append(
                    (qi, jobs[g0:g0 + GRP], g0 == 0, g0 + GRP >= len(jobs)))

        av_tiles = {}
        deferred = []

        def issue_scores(G):
            qi, grp, first, last = G
            qs = qi * QT
            sc = sp.tile([128, GRP * QT], F32, tag="sc")
            for ji, (kc, h) in enumerate(grp):
                nc.tensor.matmul(
                    sc[:, ji * QT:(ji + 1) * QT],
                    KT_sb[64 * h:64 * h + 64, kc * KC:(kc + 1) * KC],
                    QT_sb[64 * h:64 * h + 64, qs:qs + QT],
                    start=True, stop=True,
                )
            return sc

        def issue_rest(G, sc):
            qi, grp, first, last = G
            qs = qi * QT
            nsteps = (qi + 1) * QT // KC
            if first:
                av_tiles[qi] = avp.tile([128, 2 * QT], F32, tag="av", name="av")
            av = av_tiles[qi]
            width = len(grp) * QT
            pt = ptp.tile([128, GRP * QT], BF16, tag="pt")
            nc.scalar.activation(
                pt[:, :width], sc[:, :width], AF.Exp,
                scale=1.0 / math.sqrt(HEAD_DIM),
            )
            for ji, (kc, h) in enumerate(grp):
                ptj = pt[:, ji * QT:(ji + 1) * QT]
                if kc >= nsteps - NPAT:  # diagonal straddle
                    pat = kc - (nsteps - NPAT)
                    m = mask_sb[:, pat * QT:(pat + 1) * QT]
                    nc.vector.tensor_mul(ptj, ptj, m)
                nc.tensor.matmul(
                    av[:, h * QT:(h + 1) * QT],
                    V_sb[:, kc * VST + 128 * h:kc * VST + 128 * h + 128],
                    ptj, start=(kc == 0), stop=(kc == nsteps - 1),
                )
            if deferred:
                deferred.pop(0)()
            if not last:
                return
            # unnormalized per-slot out-projection; host divides by the
            # denominators (flash-attention-style partial combination).
            # The 12 (matmul+cast+dma) units are deferred and spread one per
            # subsequent score group so the DVE casts never block the masks.
            av = av_tiles.pop(qi)
            hvA = hvp.tile([128, QT], BF16, tag="hvA", name="hvA")
            nc.vector.tensor_copy(hvA[:], av[:, 0:QT])
            hvB = hvp.tile([128, QT], BF16, tag="hvB", name="hvB")
            nc.vector.tensor_copy(hvB[:], av[:, QT:2 * QT])
            nc.sync.dma_start(out=denA_d[0:1, qs:qs + QT], in_=hvA[64:65, :])
            nc.sync.dma_start(out=denB_d[0:1, qs:qs + QT], in_=hvB[0:1, :])

            def mk_op(dch, hv, rows, out_d, qs=qs):
                def emit():
                    op = sp.tile([128, QT], F32, tag="sc", name="op")
                    nc.tensor.matmul(
                        op[:], wo_sb[rows, dch * 128:(dch + 1) * 128], hv[rows, :],
                        start=True, stop=True,
                    )
                    ot = osb.tile([128, QT], BF16, tag="ot", name="ot")
                    nc.vector.tensor_copy(ot[:], op[:])
                    nc.sync.dma_start(
                        out=out_d[dch * 128:(dch + 1) * 128, qs:qs + QT], in_=ot[:],
                    )
                return emit
            for dch in range(CCH):
                deferred.append(mk_op(dch, hvA, slice(0, 64), outA_d))
                deferred.append(mk_op(dch, hvB, slice(64, 128), outB_d))

        from collections import deque
        pend = deque()
        for G in allgroups:
            sc = issue_scores(G)
            pend.append((G, sc))
            if len(pend) > 2:
                issue_rest(*pend.popleft())
        while pend:
            issue_rest(*pend.popleft())
        while deferred:
            deferred.pop(0)()
    nc.finalize()
    return nc


def _host_inputs(x, wq, bq, wk, bk, wv, bv, wo):
    """Per-core input maps. Slot A of core c = head c; slot B = head 8+c
    (cores 0-3) or a dummy zero head (cores 4-7)."""
    bf16 = ml_dtypes.bfloat16
    xT = np.ascontiguousarray(x[0].T).astype(bf16)
    masks = np.zeros((NPAT, 128, QT), np.float32)
    dk = np.arange(128)[:, None]
    dq = np.arange(QT)[None, :]
    for p in range(NPAT):
        masks[p] = (dk + 128 * p <= dq)
    masks = masks.astype(bf16)
    ident = np.eye(128, dtype=np.float32).astype(bf16)

    in_maps = []
    for c in range(N_CORES):
        hA = c
        hB = 8 + c if c < 4 else None
        w = np.zeros((3, D_MODEL, 128), np.float32)
        b = np.zeros((128, 3), np.float32)
        wo2 = np.zeros((128, D_MODEL), np.float32)
        for s, (W, B) in enumerate(((wq, bq), (wk, bk), (wv, bv))):
            w[s, :, 0:64] = W[hA]
            b[0:64, s] = B[hA]
            if hB is not None:
                w[s, :, 64:128] = W[hB]
                b[64:128, s] = B[hB]
        wo2[0:64, :] = wo[hA * 64:(hA + 1) * 64, :]
        if hB is not None:
            wo2[64:128, :] = wo[hB * 64:(hB + 1) * 64, :]
        in_maps.append({
            "xT": xT,
            "wqkv": w.astype(bf16),
            "bqkv": b.astype(np.float32),
            "wo2": wo2.astype(bf16),
            "masks": masks,
            "ident": ident,
        })
    return in_maps


def kernel(_trace=False, _tmpdir=None, **inputs):
    x = np.asarray(inputs["x"], np.float32)
    args = (x,
            np.asarray(inputs["wq"], np.float32), np.asarray(inputs["bq"], np.float32),
            np.asarray(inputs["wk"], np.float32), np.asarray(inputs["bk"], np.float32),
            np.asarray(inputs["wv"], np.float32), np.asarray(inputs["bv"], np.float32),
            np.asarray(inputs["wo"], np.float32))
    bo = np.asarray(inputs["bo"], np.float32)

    if "nc" not in _PROGRAM_CACHE:
        _PROGRAM_CACHE["nc"] = build_program()
    nc = _PROGRAM_CACHE["nc"]

    in_maps = _host_inputs(*args)
    res = run_bass_kernel_spmd(
        nc, in_maps, list(range(N_CORES)), trace=_trace, tmpdir=_tmpdir,
    )
    acc = np.zeros((D_MODEL, T), np.float32)
    for c in range(N_CORES):
        r = res.results[c]
        acc += r["outA"].astype(np.float32) / r["denA"].astype(np.float32)
        acc += r["outB"].astype(np.float32) / r["denB"].astype(np.float32)
    out = acc.T + bo[None, :]
    if _trace:
        return out[None].astype(np.float32), res
    return out[None].astype(np.float32)
